# revision 12
# baseline (speedup 1.0000x reference)
"""Masked self-attention (softmax over axis=1) Bass kernel for TRN2, 8 cores.

Reference semantics (per batch b):
    attn[l, m] = <a_l, a_m> * temperature            [L, L]
    attn = where(mask[l, m], attn, -1e7)
    P = softmax(attn, axis=l)                        (softmax over dim 0)
    out[m, :] = sum_l P[l, m] * a[l, :]              [L, H]

Equivalently out = softmax_cols(masked scores)^T @ a. We compute, per core
(4 batches each, pure data parallel across 8 cores):

    AT = a^T (bf16, via DMA-xbar transpose through a DRAM bounce)
    S[l, m] tile = sum_d AT[d, l-tile]^T @ AT[d, m-chunk]     (PE, bf16)
    S' = mask * (BIG/temp) + S                                 (DVE, one pass)
    E = exp(temp * S' - BIG)  -> bf16                          (ACT, one pass)
        (= exp(temp*S) where kept, ~e^-50 where masked: effectively 0)
    [feat | denom] = E^T @ [a | 1]                             (PE, bf16)
    out = feat * (1/denom)                                     (DVE recip + ACT scale)

No max-subtraction needed: scores*temp ~ N(0,1), diagonal ~ +28, exp stays
well inside fp32 range; denominators accumulate in fp32 PSUM.
"""

import sys

import numpy as np

sys.path.insert(0, "/opt/trn_rl_repo")

B, L, H = 32, 1024, 768
N_CORES = 8
B_LOCAL = B // N_CORES  # 4 batches per core
LT = L // 128  # 8 l-tiles
DT = H // 128  # 6 d-tiles
BIG = 50.0

_CACHE = {}


def _build(temp: float, repeats: int = 1, bench: bool = False):
    from contextlib import ExitStack

    import concourse.bass as bass
    import concourse.mybir as mybir
    from concourse import bacc, tile

    f32 = mybir.dt.float32
    bf16 = mybir.dt.bfloat16
    u8 = mybir.dt.uint8

    nc = bacc.Bacc(
        "TRN2", target_bir_lowering=False, debug=False, num_devices=N_CORES
    )

    if bench:
        # Timing-only variant: big tensors live in Internal DRAM (content
        # irrelevant — instruction stream is identical), so per-call axon
        # transfer overhead stays tiny and the R-repeat delta is clean.
        nc.dram_tensor("bench_in", [1, 4], f32, kind="ExternalInput")
        nc.dram_tensor("out", [1, 4], f32, kind="ExternalOutput")
        a_ext = nc.dram_tensor("a", [B_LOCAL, L, H], f32).ap()
        m_ext = nc.dram_tensor("mask_a", [B_LOCAL, L, L], u8).ap()
        out_ext = nc.dram_tensor("out_int", [B_LOCAL, L, H], f32).ap()
    else:
        a_ext = nc.dram_tensor("a", [B_LOCAL, L, H], f32, kind="ExternalInput").ap()
        m_ext = nc.dram_tensor(
            "mask_a", [B_LOCAL, L, L], u8, kind="ExternalInput"
        ).ap()
        out_ext = nc.dram_tensor(
            "out", [B_LOCAL, L, H], f32, kind="ExternalOutput"
        ).ap()

    big_over_temp = BIG / temp

    with tile.TileContext(nc) as tc, ExitStack() as ctx:
        t2_pool = ctx.enter_context(tc.tile_pool(name="t2", bufs=2))
        at_pool = ctx.enter_context(tc.tile_pool(name="at", bufs=2))
        mask_pool = ctx.enter_context(tc.tile_pool(name="mask", bufs=2))
        e_pool = ctx.enter_context(tc.tile_pool(name="e", bufs=2))
        sp_pool = ctx.enter_context(tc.tile_pool(name="sp", bufs=4))
        out_pool = ctx.enter_context(tc.tile_pool(name="outp", bufs=3))
        rc_pool = ctx.enter_context(tc.tile_pool(name="rc", bufs=3))
        dram_pool = ctx.enter_context(
            tc.tile_pool(name="bounce", bufs=2, space="DRAM")
        )
        psum_s = ctx.enter_context(tc.tile_pool(name="ps_s", bufs=3, space="PSUM"))
        psum_o = ctx.enter_context(tc.tile_pool(name="ps_o", bufs=2, space="PSUM"))
        const_pool = ctx.enter_context(tc.tile_pool(name="const", bufs=1))

        neg_big = const_pool.tile([128, 1], f32)
        nc.vector.memset(neg_big[:], -BIG)

        for b in [b for _ in range(repeats) for b in range(B_LOCAL)]:
            a_v = a_ext[b].rearrange("(i p) d -> p i d", p=128)  # [128, 8, 768]
            m_v = m_ext[b].rearrange("(i p) m -> p i m", p=128)  # [128, 8, 1024]
            o_v = out_ext[b].rearrange("(i p) d -> p i d", p=128)

            # a (f32, HBM) --cast--> [128, 8, 769] bf16 with a ones column.
            t2 = t2_pool.tile([128, LT, H + 1], bf16)
            nc.gpsimd.dma_start(out=t2[:, :, 0:H], in_=a_v)
            nc.vector.memset(t2[:, :, H : H + 1], 1.0)

            msk = mask_pool.tile([128, LT, L], u8)
            nc.sync.dma_start(out=msk[:], in_=m_v)

            # bf16 copy of a to DRAM, then 6 xbar-transpose DMAs -> AT [768, L]
            scratch = dram_pool.tile([L, H], bf16)
            nc.sync.dma_start(
                out=scratch[:].rearrange("(i p) d -> p i d", p=128),
                in_=t2[:, :, 0:H],
            )
            at = at_pool.tile([128, DT, L], bf16)
            for j in range(DT):
                nc.sync.dma_start(
                    out=at[:, j, :],
                    in_=scratch[:, 128 * j : 128 * (j + 1)],
                    transpose=True,
                )

            # E[l, m] = exp(temp*S + (mask-1)*BIG), bf16, [128, 8, 1024]
            e = e_pool.tile([128, LT, L], bf16)
            for li in range(LT):
                for c in range(2):
                    ps = psum_s.tile([128, 512], f32)
                    for j in range(DT):
                        nc.tensor.matmul(
                            ps[:],
                            lhsT=at[:, j, 128 * li : 128 * (li + 1)],
                            rhs=at[:, j, 512 * c : 512 * (c + 1)],
                            start=(j == 0),
                            stop=(j == DT - 1),
                        )
                    sp = sp_pool.tile([128, 512], f32)
                    nc.vector.scalar_tensor_tensor(
                        out=sp[:],
                        in0=msk[:, li, 512 * c : 512 * (c + 1)],
                        scalar=big_over_temp,
                        in1=ps[:],
                        op0=mybir.AluOpType.mult,
                        op1=mybir.AluOpType.add,
                    )
                    nc.scalar.activation(
                        out=e[:, li, 512 * c : 512 * (c + 1)],
                        in_=sp[:],
                        func=mybir.ActivationFunctionType.Exp,
                        bias=neg_big[:],
                        scale=temp,
                    )

            # [feat | denom] = E^T @ [a | 1]; normalize; store.
            for mi in range(LT):
                po = psum_o.tile([128, H + 1], f32)
                for li in range(LT):
                    w = e[:, li, 128 * mi : 128 * (mi + 1)]
                    nc.tensor.matmul(
                        po[:, 0:512],
                        lhsT=w,
                        rhs=t2[:, li, 0:512],
                        start=(li == 0),
                        stop=(li == LT - 1),
                    )
                    nc.tensor.matmul(
                        po[:, 512 : H + 1],
                        lhsT=w,
                        rhs=t2[:, li, 512 : H + 1],
                        start=(li == 0),
                        stop=(li == LT - 1),
                    )
                rc = rc_pool.tile([128, 1], f32)
                nc.vector.reciprocal(rc[:], po[:, H : H + 1])
                ot = out_pool.tile([128, H], f32)
                nc.scalar.activation(
                    out=ot[:],
                    in_=po[:, 0:H],
                    func=mybir.ActivationFunctionType.Copy,
                    scale=rc[:],
                )
                nc.sync.dma_start(out=o_v[:, mi, :], in_=ot[:])

    nc.compile()
    return nc


def _get_nc(temp: float, repeats: int = 1, bench: bool = False):
    key = (round(float(temp), 12), repeats, bench)
    if key not in _CACHE:
        _CACHE[key] = _build(float(temp), repeats, bench)
    return _CACHE[key]


def run(a, mask_a, temperature=None, trace=False):
    from concourse.bass_utils import run_bass_kernel_spmd

    a = np.ascontiguousarray(np.asarray(a, dtype=np.float32))
    mask_u8 = np.ascontiguousarray(np.asarray(mask_a)).view(np.uint8)
    if temperature is None:
        temperature = 1.0 / np.sqrt(np.float32(H))
    temp = float(np.asarray(temperature, dtype=np.float32))

    nc = _get_nc(temp)
    in_maps = [
        {
            "a": a[c * B_LOCAL : (c + 1) * B_LOCAL],
            "mask_a": mask_u8[c * B_LOCAL : (c + 1) * B_LOCAL],
        }
        for c in range(N_CORES)
    ]
    res = run_bass_kernel_spmd(
        nc, in_maps, core_ids=list(range(N_CORES)), trace=trace
    )
    out = np.concatenate([res.results[c]["out"] for c in range(N_CORES)], axis=0)
    return out, res


def kernel(a, mask_a, temperature=None, **_):
    out, _res = run(a, mask_a, temperature)
    return out
